# revision 23
# baseline (speedup 1.0000x reference)
"""Grok1 MoE (unfused) Trainium2 Bass kernel — sparse top-2 routing.

Expert-parallel over 8 NeuronCores: core e owns expert e's w1/w3/w2.
Only top-2 of 8 experts matter per token, so each core processes just
the ~T*2/8 tokens routed to its expert instead of all T (4x fewer
matmul FLOPs than the dense formulation).

Host: computes the (tiny) router in fp32, gathers each expert's tokens
into a padded token slab, and scatter-adds the per-expert outputs back
into the full [T, H] result.
Device (per core): y = (gelu(x@w1.T) * (x@w3.T)) @ w2.T on its C-token
slab, scaled by the combine weight, fp16 matmuls with fp32 PSUM.

Perf structure (from NTFF trace analysis; the kernel is PE-bound at
~86% MFU so the wins are at the edges of the matmul stream):
- weights pre-transposed on host into exact SBUF tile layout so every
  weight DMA is one fully-contiguous block,
- startup: x slabs are split into 2-h-chunk tiles spread over FOUR DMA
  queues (SP/Activation/GpSimd/DVE), interleaved with the first w1/w3
  chunk (split in half for earlier arrival), so the first real matmul
  issues ~1.5us after the DMA queues open instead of ~6us,
- w1/w3 stream just-in-time on the SP/ACT rings (bufs=4 prefetch);
  ALL of w2 preloads on the otherwise-idle GpSimd ring during phase 1,
  so phase 2 never waits on DMA,
- token dim padded only to a multiple of 16 and split into near-equal
  PSUM-bank-sized blocks (no 128-wide tail matmuls),
- 6 short warmup matmuls bridge the PE p-state ramp until x lands,
- the final eviction is quartered across two DMA queues so the tail
  drains ~2us sooner.
"""

import numpy as np

import concourse.bass as bass
import concourse.mybir as mybir
import concourse.tile as tile
from concourse import bacc
from concourse.bass import ts
from concourse.bass_utils import run_bass_kernel_spmd

T, H, F, E = 2048, 1024, 4096, 8
NCORES = 8
HC = H // 128   # 8 h-chunks
FC = F // 128   # 32 f-chunks
NWARM = 14      # PE p-state warmup matmuls (bridge until x + first weights land)

f32 = mybir.dt.float32
f16 = mybir.dt.float16

_CACHE = {}


def _blocks(C):
    """Split C tokens into near-equal blocks of <=512 (PSUM bank limit),
    each a multiple of 16 except possibly the last."""
    nblk = -(-C // 512)
    base = -(-C // nblk)
    base = -(-base // 16) * 16
    blks = []
    off = 0
    while off < C:
        sz = min(base, C - off)
        blks.append((off, sz))
        off += sz
    return blks


def build_nc(C):
    assert C % 16 == 0 and C <= T
    blks = _blocks(C)
    nblk = len(blks)

    nc = bacc.Bacc(
        "TRN2",
        target_bir_lowering=False,
        debug=False,
        num_devices=NCORES,
    )

    # block 0 ships as two partition-major halves (h0-3 / h4-7) so the two
    # startup x DMAs are big AND fully contiguous; later blocks ship whole.
    xg0d = [
        nc.dram_tensor(f"xg0{s}", [128, 4, blks[0][1]], f16, kind="ExternalInput")
        for s in "ab"
    ]
    xgs = [
        nc.dram_tensor(f"xg{i}", [128, HC, sz], f16, kind="ExternalInput")
        for i, (_, sz) in enumerate(blks)
        if i > 0
    ]
    cb = nc.dram_tensor("cb", [C], f32, kind="ExternalInput")
    # f=0/f=1 ship as single contiguous chunks for the startup race; the
    # rest of the w1/w3 streams ship as f-PAIRS laid out pair-major so each
    # stream DMA moves 4KB-per-partition runs (2KB runs halve DMA efficiency)
    w1f0d = nc.dram_tensor("w1f0d", [128, HC, 128], f16, kind="ExternalInput")
    w1f1d = nc.dram_tensor("w1f1d", [128, HC, 128], f16, kind="ExternalInput")
    w3f0d = nc.dram_tensor("w3f0d", [128, HC, 128], f16, kind="ExternalInput")
    w3f1d = nc.dram_tensor("w3f1d", [128, HC, 128], f16, kind="ExternalInput")
    NP = (FC - 2) // 2
    w1pp = nc.dram_tensor("w1pp", [NP, 128, 2, HC, 128], f16, kind="ExternalInput")
    w3pp = nc.dram_tensor("w3pp", [NP, 128, 2, HC, 128], f16, kind="ExternalInput")
    w2p = nc.dram_tensor("w2p", [HC, 2, 128, FC // 2, 128], f16, kind="ExternalInput")
    out = nc.dram_tensor("out", [H, C], f32, kind="ExternalOutput")

    AF = mybir.ActivationFunctionType

    with tile.TileContext(nc) as tc:
        with (
            tc.tile_pool(name="big", bufs=1) as big,
            tc.tile_pool(name="singles", bufs=1) as singles,
            tc.tile_pool(name="wpool", bufs=4) as wpool,
            tc.tile_pool(name="w2pool", bufs=1) as w2pool,
            tc.tile_pool(name="evict", bufs=3) as evict,
            tc.tile_pool(name="psum_w", bufs=2, space="PSUM") as psum_w,
            tc.tile_pool(name="psum_gu", bufs=2, space="PSUM") as psum_gu,
            tc.tile_pool(name="psum_o", bufs=2, space="PSUM") as psum_o,
        ):
            # ---- PE warmup: ramp the p-state while inputs stream in ----
            # memset on GpSimd so the DVE/ACT/SP queues open with x DMAs.
            warm = singles.tile([128, 512], f16, tag="warm")
            nc.gpsimd.memset(warm, 0.0)
            for _ in range(NWARM):
                wps = psum_w.tile([128, 512], f32, tag="wm")
                nc.tensor.matmul(wps, lhsT=warm[:, :128], rhs=warm, start=True, stop=True)

            # ---- input loads ----
            # Few, big, contiguous DMAs (the queues have ~1us per-DMA
            # overhead, so fine-grained chunking backfires):
            #   SP:     x0 h0-3 half, then the w1 stream (f>=1) + w2 evens
            #   ACT:    w1f0, w3f0, dummy-gelu, w3 stream + w2 odds
            #   GpSimd: warm memset, x0 h4-7 half, block-1 x, cb
            # The dummy gelu forces the scalar engine's lazy activation-
            # table load NOW (1.3us) instead of stalling the w3 stream at
            # the first real gelu. Block 1 of f=0 is deferred to the END
            # of phase 1 (the f0 weight tiles stay resident), keeping
            # block-1 x off the critical path.
            x0h = []
            for s, eng in (("a", nc.sync), ("b", nc.gpsimd)):
                t = singles.tile([128, 4, blks[0][1]], f16, tag=f"x0{s}", name=f"x0{s}")
                eng.dma_start(out=t, in_=xg0d[0 if s == "a" else 1].ap())
                x0h.append(t)
            w1f0 = wpool.tile([128, HC, 128], f16, tag="w1f0", name="w1f0")
            nc.scalar.dma_start(out=w1f0, in_=w1f0d.ap())
            w3f0 = wpool.tile([128, HC, 128], f16, tag="w3f0", name="w3f0")
            nc.scalar.dma_start(out=w3f0, in_=w3f0d.ap())
            w1f1 = wpool.tile([128, HC, 128], f16, tag="w1f1", name="w1f1")
            nc.sync.dma_start(out=w1f1, in_=w1f1d.ap())
            w3f1 = wpool.tile([128, HC, 128], f16, tag="w3f1", name="w3f1")
            nc.sync.dma_start(out=w3f1, in_=w3f1d.ap())
            # dummy activation: preload the gelu table while the PE warms up
            gdummy = singles.tile([128, 16], f32, tag="gdummy")
            nc.scalar.activation(gdummy, warm[:, :16], AF.Gelu)

            xb = [None] * nblk
            for b in range(1, nblk):
                t = singles.tile([128, HC, blks[b][1]], f16, tag=f"xb{b}", name=f"xb{b}")
                nc.gpsimd.dma_start(out=t, in_=xgs[b - 1].ap())
                xb[b] = t

            cb_b = singles.tile([128, C], f32, tag="cb")
            cb_src = bass.AP(tensor=cb.ap().tensor, offset=0, ap=[[0, 128], [1, C]])
            nc.gpsimd.dma_start(out=cb_b, in_=cb_src)

            def x_rhs(bi, h):
                if bi == 0:
                    return x0h[h // 4][:, h % 4, :]
                return xb[bi][:, h, :]

            # ---- phase 1: gus = gelu(w1 @ x) * (w3 @ x), [F-part, tokens] ----
            w2sb = {}
            w2chunks = [(h, hv) for h in range(HC) for hv in range(2)]
            gus = big.tile([128, FC, C], f16)

            def gu_group(f, bi, w1sel, w3sel):
                off, sz = blks[bi]
                g_ps = psum_gu.tile([128, 512], f32, tag="g", name="g_ps")
                for h in range(HC):
                    nc.tensor.matmul(
                        g_ps[:, :sz],
                        lhsT=w1sel(h),
                        rhs=x_rhs(bi, h),
                        start=(h == 0),
                        stop=(h == HC - 1),
                    )
                u_ps = psum_gu.tile([128, 512], f32, tag="u", name="u_ps")
                for h in range(HC):
                    nc.tensor.matmul(
                        u_ps[:, :sz],
                        lhsT=w3sel(h),
                        rhs=x_rhs(bi, h),
                        start=(h == 0),
                        stop=(h == HC - 1),
                    )
                gs = evict.tile([128, 512], f32, tag="gs", name="gs")
                nc.scalar.activation(gs[:, :sz], g_ps[:, :sz], AF.Gelu)
                nc.vector.tensor_mul(
                    gus[:, f, off : off + sz], gs[:, :sz], u_ps[:, :sz]
                )

            w1sel0 = lambda h: w1f0[:, h, :]
            w3sel0 = lambda h: w3f0[:, h, :]
            w1t = w3t = None
            w1sel1 = lambda h: w1f1[:, h, :]
            w3sel1 = lambda h: w3f1[:, h, :]
            for f in range(FC):
                if f == 0:
                    gu_group(0, 0, w1sel0, w3sel0)
                    continue
                if f == 1:
                    gu_group(1, 0, w1sel1, w3sel1)
                    continue
                if True:
                    if (f - 2) % 2 == 0:
                        pp = (f - 2) // 2
                        w1t = wpool.tile([128, 2, HC, 128], f16, tag="w1f", name="w1t")
                        nc.sync.dma_start(out=w1t, in_=w1pp.ap()[pp])
                        w3t = wpool.tile([128, 2, HC, 128], f16, tag="w3f", name="w3t")
                        nc.scalar.dma_start(out=w3t, in_=w3pp.ap()[pp])
                    j = (f - 2) % 2
                    w1sel = lambda h, t=w1t, j=j: t[:, j, h, :]
                    w3sel = lambda h, t=w3t, j=j: t[:, j, h, :]
                if f == 10:
                    # Gate the whole w2 preload behind a w1-pair buffer reuse:
                    # this dummy fetch can only issue once the f=2/3 pair tile
                    # retires (~16us), so the 8MB w2 burst on the GpSimd ring
                    # stays clear of the startup race, then runs at full rate.
                    wdelay = wpool.tile([128, 2, HC, 128], f16, tag="w1f", name="wdelay")
                    nc.gpsimd.dma_start(out=wdelay, in_=w1pp.ap()[0])
                    for h2, hv2 in w2chunks:
                        t = w2pool.tile([128, FC // 2, 128], f16, tag=f"w2_{h2}_{hv2}")
                        nc.gpsimd.dma_start(out=t, in_=w2p.ap()[h2, hv2])
                        w2sb[h2, hv2] = t
                for bi in range(nblk):
                    gu_group(f, bi, w1sel, w3sel)
            for bi in range(1, nblk):
                gu_group(0, bi, w1sel0, w3sel0)
                gu_group(1, bi, w1sel1, w3sel1)

            # ---- phase 2: out[h,:] = (w2 @ gus) * cb (w2 already in SBUF) ----
            for bi in range(nblk):
                off, sz = blks[bi]
                for h in range(HC):
                    o_ps = psum_o.tile([128, 512], f32, tag="o", name="o_ps")
                    for f in range(FC):
                        nc.tensor.matmul(
                            o_ps[:, :sz],
                            lhsT=w2sb[h, f // (FC // 2)][:, f % (FC // 2), :],
                            rhs=gus[:, f, off : off + sz],
                            start=(f == 0),
                            stop=(f == FC - 1),
                        )
                    o_sb = evict.tile([128, 512], f32, tag="osb", name="o_sb")
                    # halve the very last eviction across the two DMA queues so
                    # the final out-DMAs drain in parallel with the multiplies
                    last = h == HC - 1 and bi == nblk - 1
                    if last:
                        pieces = [(0, sz // 2), (sz // 2, sz - sz // 2)]
                    else:
                        pieces = [(0, sz)]
                    for k, (po, pw) in enumerate(pieces):
                        nc.vector.tensor_mul(
                            o_sb[:, po : po + pw],
                            o_ps[:, po : po + pw],
                            cb_b[:, off + po : off + po + pw],
                        )
                        eng = nc.sync if (bi + k) % 2 == 0 else nc.scalar
                        eng.dma_start(
                            out=out.ap()[ts(h, 128), off + po : off + po + pw],
                            in_=o_sb[:, po : po + pw],
                        )
    nc.finalize()
    return nc


def _route(hidden_states, gate_w):
    """Exact fp32 router matching the reference: softcap -> softmax -> top2."""
    hs = hidden_states.astype(np.float32)
    logits = hs @ gate_w.T.astype(np.float32)
    logits = 30.0 * np.tanh(logits / 30.0)
    lmax = logits.max(axis=-1, keepdims=True)
    p = np.exp(logits - lmax)
    probs = p / p.sum(axis=-1, keepdims=True)
    idx = np.argsort(-probs, axis=-1, kind="stable")[:, :2]
    vals = np.take_along_axis(probs, idx, axis=-1)
    return idx, vals


def _prep_weights(w1, w2, w3):
    """Pre-transpose expert weights into contiguous SBUF tile layouts."""
    wmaps = []
    for e in range(NCORES):
        # w1p[f, p, c, m] = w1[e][f*128+m, c*128+p]
        w1p = np.ascontiguousarray(
            w1[e].astype(np.float16).reshape(FC, 128, HC, 128).transpose(0, 3, 2, 1)
        )
        w3p = np.ascontiguousarray(
            w3[e].astype(np.float16).reshape(FC, 128, HC, 128).transpose(0, 3, 2, 1)
        )
        # w2p[h, hv, p, c, m] = w2[e][h*128+m, hv*F/2 + c*128+p]
        w2p = np.ascontiguousarray(
            w2[e]
            .astype(np.float16)
            .reshape(HC, 128, 2, FC // 2, 128)
            .transpose(0, 2, 4, 3, 1)
        )
        def pairs(wp):
            # [15, 128, 2, HC, 128] pair-major from f=2 on (4KB runs/partition)
            return np.ascontiguousarray(
                wp[2:].reshape((FC - 2) // 2, 2, 128, HC, 128).transpose(0, 2, 1, 3, 4)
            )
        wmaps.append({
            "w2p": w2p,
            "w1f0d": w1p[0], "w1f1d": np.ascontiguousarray(w1p[1]),
            "w3f0d": w3p[0], "w3f1d": np.ascontiguousarray(w3p[1]),
            "w1pp": pairs(w1p), "w3pp": pairs(w3p),
        })
    return wmaps


def kernel(hidden_states, gate_w, w1, w2, w3, trace=False):
    hidden_states = np.asarray(hidden_states, dtype=np.float32)
    gate_w = np.asarray(gate_w, dtype=np.float32)
    w1 = np.asarray(w1, dtype=np.float32)
    w2 = np.asarray(w2, dtype=np.float32)
    w3 = np.asarray(w3, dtype=np.float32)

    idx, vals = _route(hidden_states, gate_w)
    toks = []
    cvals = []
    for e in range(E):
        hit = idx == e                                     # [T, 2]
        tok_e = np.where(hit.any(axis=1))[0]
        toks.append(tok_e)
        cvals.append((vals * hit)[tok_e].sum(axis=1).astype(np.float32))
    nmax = max(len(t) for t in toks)
    C = max(16, -(-nmax // 16) * 16)
    blks = _blocks(C)

    if C not in _CACHE:
        _CACHE[C] = build_nc(C)
    nc = _CACHE[C]

    xT16 = np.ascontiguousarray(hidden_states.T).astype(np.float16)  # [H, T]
    wmaps = _prep_weights(w1, w2, w3)
    in_maps = []
    for e in range(NCORES):
        n_e = len(toks[e])
        xg = np.zeros((128, HC, C), dtype=np.float16)
        # xg[p, c, :n] = x[c*128+p, toks]
        xg[:, :, :n_e] = xT16.reshape(HC, 128, T)[:, :, toks[e]].transpose(1, 0, 2)
        cbv = np.zeros((C,), dtype=np.float32)
        cbv[:n_e] = cvals[e]
        m = {"cb": cbv, **wmaps[e]}
        sz0 = blks[0][1]
        m["xg0a"] = np.ascontiguousarray(xg[:, 0:4, :sz0])
        m["xg0b"] = np.ascontiguousarray(xg[:, 4:8, :sz0])
        for i, (off, sz) in enumerate(blks):
            if i > 0:
                m[f"xg{i}"] = np.ascontiguousarray(xg[:, :, off : off + sz])
        in_maps.append(m)

    res = run_bass_kernel_spmd(nc, in_maps, core_ids=list(range(NCORES)), trace=trace)
    out = np.zeros((T, H), dtype=np.float32)
    for e in range(NCORES):
        n_e = len(toks[e])
        out[toks[e]] += res.results[e]["out"][:, :n_e].T
    _CACHE["last_results"] = res
    return out


if __name__ == "__main__":
    rng = np.random.default_rng(0)
    hs = rng.standard_normal((T, H), dtype=np.float32)
    gw = (rng.standard_normal((E, H)) * 0.02).astype(np.float32)
    w1 = (rng.standard_normal((E, F, H)) * 0.02).astype(np.float32)
    w2 = (rng.standard_normal((E, H, F)) * 0.02).astype(np.float32)
    w3 = (rng.standard_normal((E, F, H)) * 0.02).astype(np.float32)
    out = kernel(hs, gw, w1, w2, w3)
    print("out", out.shape, out.dtype, np.abs(out).max())


# revision 24
# speedup vs baseline: 1.0630x; 1.0630x over previous
"""Grok1 MoE (unfused) Trainium2 Bass kernel — sparse top-2 routing.

Expert-parallel over 8 NeuronCores: core e owns expert e's w1/w3/w2.
Only top-2 of 8 experts matter per token, so each core processes just
the ~T*2/8 tokens routed to its expert instead of all T (4x fewer
matmul FLOPs than the dense formulation).

Host: computes the (tiny) router in fp32, gathers each expert's tokens
into a padded token slab, and scatter-adds the per-expert outputs back
into the full [T, H] result.
Device (per core): y = (gelu(x@w1.T) * (x@w3.T)) @ w2.T on its C-token
slab, scaled by the combine weight, fp16 matmuls with fp32 PSUM.

Perf structure (from NTFF trace analysis):
- weights pre-transposed on host into exact SBUF tile layout so every
  weight DMA is one fully-contiguous block,
- w1/w3 stream just-in-time on the SP DMA ring (bufs=4 prefetch);
  ALL of w2 (64KB/partition) preloads on the Activation DMA ring
  during phase 1, so phase 2 never waits on DMA,
- token dim padded only to a multiple of 16 and split into near-equal
  PSUM-bank-sized blocks (no 128-wide tail matmuls),
- a few dummy warmup matmuls ramp the PE p-state while x loads,
- the final eviction's two halves drain on both DMA rings in parallel.
"""

import numpy as np

import concourse.bass as bass
import concourse.mybir as mybir
import concourse.tile as tile
from concourse import bacc
from concourse.bass import ts
from concourse.bass_utils import run_bass_kernel_spmd

T, H, F, E = 2048, 1024, 4096, 8
NCORES = 8
HC = H // 128   # 8 h-chunks
FC = F // 128   # 32 f-chunks
NWARM = 14      # PE p-state warmup matmuls (bridge until x + first weights land)

f32 = mybir.dt.float32
f16 = mybir.dt.float16

_CACHE = {}


def _blocks(C):
    """Split C tokens into near-equal blocks of <=512 (PSUM bank limit),
    each a multiple of 16 except possibly the last."""
    nblk = -(-C // 512)
    base = -(-C // nblk)
    base = -(-base // 16) * 16
    blks = []
    off = 0
    while off < C:
        sz = min(base, C - off)
        blks.append((off, sz))
        off += sz
    return blks


def build_nc(C):
    assert C % 16 == 0 and C <= T
    blks = _blocks(C)

    nc = bacc.Bacc(
        "TRN2",
        target_bir_lowering=False,
        debug=False,
        num_devices=NCORES,
    )

    xgs = [
        nc.dram_tensor(f"xg{i}", [128, HC, sz], f16, kind="ExternalInput")
        for i, (_, sz) in enumerate(blks)
    ]
    cb = nc.dram_tensor("cb", [C], f32, kind="ExternalInput")
    w1p = nc.dram_tensor("w1p", [FC, 128, HC, 128], f16, kind="ExternalInput")
    w3p = nc.dram_tensor("w3p", [FC, 128, HC, 128], f16, kind="ExternalInput")
    w2p = nc.dram_tensor("w2p", [HC, 2, 128, FC // 2, 128], f16, kind="ExternalInput")
    out = nc.dram_tensor("out", [H, C], f32, kind="ExternalOutput")

    AF = mybir.ActivationFunctionType

    with tile.TileContext(nc) as tc:
        with (
            tc.tile_pool(name="big", bufs=1) as big,
            tc.tile_pool(name="singles", bufs=1) as singles,
            tc.tile_pool(name="wpool", bufs=10) as wpool,
            tc.tile_pool(name="w2pool", bufs=1) as w2pool,
            tc.tile_pool(name="evict", bufs=3) as evict,
            tc.tile_pool(name="psum_w", bufs=2, space="PSUM") as psum_w,
            tc.tile_pool(name="psum_gu", bufs=2, space="PSUM") as psum_gu,
            tc.tile_pool(name="psum_o", bufs=2, space="PSUM") as psum_o,
        ):
            # ---- PE warmup: ramp the p-state while inputs stream in ----
            warm = singles.tile([128, 512], f16, tag="warm")
            nc.vector.memset(warm, 0.0)
            for _ in range(NWARM):
                wps = psum_w.tile([128, 512], f32, tag="wm")
                nc.tensor.matmul(wps, lhsT=warm[:, :128], rhs=warm, start=True, stop=True)

            # ---- input loads ----
            # SP ring: x block 0, then the just-in-time w1/w3 stream.
            # ACT ring: x block 1+, cb, then the whole-w2 preload.
            xb = []
            for i, (_, sz) in enumerate(blks):
                t = singles.tile([128, HC, sz], f16, tag=f"xb{i}")
                eng = nc.sync if i == 0 else nc.scalar
                eng.dma_start(out=t, in_=xgs[i].ap())
                xb.append(t)
            cb_b = singles.tile([128, C], f32, tag="cb")
            cb_src = bass.AP(tensor=cb.ap().tensor, offset=0, ap=[[0, 128], [1, C]])
            nc.scalar.dma_start(out=cb_b, in_=cb_src)

            # ---- phase 1: gus = gelu(w1 @ x) * (w3 @ x), [F-part, tokens] ----
            # The w2 preload is paced through the f-loop (one 512KB chunk per
            # two f-iters on the ACT ring) so its burst can't starve the
            # just-in-time w1/w3 stream of DMA-engine bandwidth.
            w2sb = {}
            w2chunks = [(h, hv) for h in range(HC) for hv in range(2)]
            gus = big.tile([128, FC, C], f16)
            for f in range(FC):
                w1f = wpool.tile([128, HC, 128], f16, tag="w1f")
                nc.sync.dma_start(out=w1f, in_=w1p.ap()[f])
                w3f = wpool.tile([128, HC, 128], f16, tag="w3f")
                nc.scalar.dma_start(out=w3f, in_=w3p.ap()[f])
                if f % 2 == 0 and f // 2 < len(w2chunks):
                    h2, hv2 = w2chunks[f // 2]
                    t = w2pool.tile([128, FC // 2, 128], f16, tag=f"w2_{h2}_{hv2}")
                    nc.scalar.dma_start(out=t, in_=w2p.ap()[h2, hv2])
                    w2sb[h2, hv2] = t
                for bi, (off, sz) in enumerate(blks):
                    g_ps = psum_gu.tile([128, 512], f32, tag="g")
                    for h in range(HC):
                        nc.tensor.matmul(
                            g_ps[:, :sz],
                            lhsT=w1f[:, h, :],
                            rhs=xb[bi][:, h, :],
                            start=(h == 0),
                            stop=(h == HC - 1),
                        )
                    u_ps = psum_gu.tile([128, 512], f32, tag="u")
                    for h in range(HC):
                        nc.tensor.matmul(
                            u_ps[:, :sz],
                            lhsT=w3f[:, h, :],
                            rhs=xb[bi][:, h, :],
                            start=(h == 0),
                            stop=(h == HC - 1),
                        )
                    gs = evict.tile([128, 512], f32, tag="gs")
                    nc.scalar.activation(gs[:, :sz], g_ps[:, :sz], AF.Gelu)
                    nc.vector.tensor_mul(
                        gus[:, f, off : off + sz], gs[:, :sz], u_ps[:, :sz]
                    )

            # ---- phase 2: out[h,:] = (w2 @ gus) * cb (w2 already in SBUF) ----
            for h in range(HC):
                for off, sz in blks:
                    o_ps = psum_o.tile([128, 512], f32, tag="o")
                    for f in range(FC):
                        nc.tensor.matmul(
                            o_ps[:, :sz],
                            lhsT=w2sb[h, f // (FC // 2)][:, f % (FC // 2), :],
                            rhs=gus[:, f, off : off + sz],
                            start=(f == 0),
                            stop=(f == FC - 1),
                        )
                    o_sb = evict.tile([128, 512], f32, tag="osb")
                    # split the last iteration's evict so the final out-DMA
                    # overlaps the final multiply instead of chaining after it
                    halves = (
                        [(0, sz // 2), (sz // 2, sz - sz // 2)]
                        if (h == HC - 1 and (off, sz) == blks[-1])
                        else [(0, sz)]
                    )
                    for hk, (ho, hs_) in enumerate(halves):
                        nc.vector.tensor_mul(
                            o_sb[:, ho : ho + hs_],
                            o_ps[:, ho : ho + hs_],
                            cb_b[:, off + ho : off + ho + hs_],
                        )
                        eng = nc.scalar if hk == 1 else nc.sync
                        eng.dma_start(
                            out=out.ap()[ts(h, 128), off + ho : off + ho + hs_],
                            in_=o_sb[:, ho : ho + hs_],
                        )
    nc.finalize()
    return nc


def _route(hidden_states, gate_w):
    """Exact fp32 router matching the reference: softcap -> softmax -> top2."""
    hs = hidden_states.astype(np.float32)
    logits = hs @ gate_w.T.astype(np.float32)
    logits = 30.0 * np.tanh(logits / 30.0)
    lmax = logits.max(axis=-1, keepdims=True)
    p = np.exp(logits - lmax)
    probs = p / p.sum(axis=-1, keepdims=True)
    idx = np.argsort(-probs, axis=-1, kind="stable")[:, :2]
    vals = np.take_along_axis(probs, idx, axis=-1)
    return idx, vals


def _prep_weights(w1, w2, w3):
    """Pre-transpose expert weights into contiguous SBUF tile layouts."""
    wmaps = []
    for e in range(NCORES):
        # w1p[f, p, c, m] = w1[e][f*128+m, c*128+p]
        w1p = np.ascontiguousarray(
            w1[e].astype(np.float16).reshape(FC, 128, HC, 128).transpose(0, 3, 2, 1)
        )
        w3p = np.ascontiguousarray(
            w3[e].astype(np.float16).reshape(FC, 128, HC, 128).transpose(0, 3, 2, 1)
        )
        # w2p[h, hv, p, c, m] = w2[e][h*128+m, hv*F/2 + c*128+p]
        w2p = np.ascontiguousarray(
            w2[e]
            .astype(np.float16)
            .reshape(HC, 128, 2, FC // 2, 128)
            .transpose(0, 2, 4, 3, 1)
        )
        wmaps.append({"w1p": w1p, "w3p": w3p, "w2p": w2p})
    return wmaps


def kernel(hidden_states, gate_w, w1, w2, w3, trace=False):
    hidden_states = np.asarray(hidden_states, dtype=np.float32)
    gate_w = np.asarray(gate_w, dtype=np.float32)
    w1 = np.asarray(w1, dtype=np.float32)
    w2 = np.asarray(w2, dtype=np.float32)
    w3 = np.asarray(w3, dtype=np.float32)

    idx, vals = _route(hidden_states, gate_w)
    toks = []
    cvals = []
    for e in range(E):
        hit = idx == e                                     # [T, 2]
        tok_e = np.where(hit.any(axis=1))[0]
        toks.append(tok_e)
        cvals.append((vals * hit)[tok_e].sum(axis=1).astype(np.float32))
    nmax = max(len(t) for t in toks)
    C = max(16, -(-nmax // 16) * 16)
    blks = _blocks(C)

    if C not in _CACHE:
        _CACHE[C] = build_nc(C)
    nc = _CACHE[C]

    xT16 = np.ascontiguousarray(hidden_states.T).astype(np.float16)  # [H, T]
    wmaps = _prep_weights(w1, w2, w3)
    in_maps = []
    for e in range(NCORES):
        n_e = len(toks[e])
        xg = np.zeros((128, HC, C), dtype=np.float16)
        # xg[p, c, :n] = x[c*128+p, toks]
        xg[:, :, :n_e] = xT16.reshape(HC, 128, T)[:, :, toks[e]].transpose(1, 0, 2)
        cbv = np.zeros((C,), dtype=np.float32)
        cbv[:n_e] = cvals[e]
        m = {"cb": cbv, **wmaps[e]}
        for i, (off, sz) in enumerate(blks):
            m[f"xg{i}"] = np.ascontiguousarray(xg[:, :, off : off + sz])
        in_maps.append(m)

    res = run_bass_kernel_spmd(nc, in_maps, core_ids=list(range(NCORES)), trace=trace)
    out = np.zeros((T, H), dtype=np.float32)
    for e in range(NCORES):
        n_e = len(toks[e])
        out[toks[e]] += res.results[e]["out"][:, :n_e].T
    _CACHE["last_results"] = res
    return out


if __name__ == "__main__":
    rng = np.random.default_rng(0)
    hs = rng.standard_normal((T, H), dtype=np.float32)
    gw = (rng.standard_normal((E, H)) * 0.02).astype(np.float32)
    w1 = (rng.standard_normal((E, F, H)) * 0.02).astype(np.float32)
    w2 = (rng.standard_normal((E, H, F)) * 0.02).astype(np.float32)
    w3 = (rng.standard_normal((E, F, H)) * 0.02).astype(np.float32)
    out = kernel(hs, gw, w1, w2, w3)
    print("out", out.shape, out.dtype, np.abs(out).max())


# revision 25
# speedup vs baseline: 1.0631x; 1.0001x over previous
"""Grok1 MoE (unfused) Trainium2 Bass kernel — sparse top-2 routing.

Expert-parallel over 8 NeuronCores: core e owns expert e's w1/w3/w2.
Only top-2 of 8 experts matter per token, so each core processes just
the ~T*2/8 tokens routed to its expert instead of all T (4x fewer
matmul FLOPs than the dense formulation).

Host: computes the (tiny) router in fp32, gathers each expert's tokens
into a padded token slab, and scatter-adds the per-expert outputs back
into the full [T, H] result.
Device (per core): y = (gelu(x@w1.T) * (x@w3.T)) @ w2.T on its C-token
slab, scaled by the combine weight, fp16 matmuls with fp32 PSUM.

Perf structure (from NTFF trace analysis):
- weights pre-transposed on host into exact SBUF tile layout so every
  weight DMA is one fully-contiguous block,
- w1/w3 stream just-in-time on the SP DMA ring (bufs=4 prefetch);
  ALL of w2 (64KB/partition) preloads on the Activation DMA ring
  during phase 1, so phase 2 never waits on DMA,
- token dim padded only to a multiple of 16 and split into near-equal
  PSUM-bank-sized blocks (no 128-wide tail matmuls),
- a few dummy warmup matmuls ramp the PE p-state while x loads,
- the final eviction's two halves drain on both DMA rings in parallel.
"""

import numpy as np

import concourse.bass as bass
import concourse.mybir as mybir
import concourse.tile as tile
from concourse import bacc
from concourse.bass import ts
from concourse.bass_utils import run_bass_kernel_spmd

T, H, F, E = 2048, 1024, 4096, 8
NCORES = 8
HC = H // 128   # 8 h-chunks
FC = F // 128   # 32 f-chunks
NWARM = 22      # PE p-state warmup matmuls (bridge until x + first weights land)

f32 = mybir.dt.float32
f16 = mybir.dt.float16

_CACHE = {}


def _blocks(C):
    """Split C tokens into near-equal blocks of <=512 (PSUM bank limit),
    each a multiple of 16 except possibly the last."""
    nblk = -(-C // 512)
    base = -(-C // nblk)
    base = -(-base // 16) * 16
    blks = []
    off = 0
    while off < C:
        sz = min(base, C - off)
        blks.append((off, sz))
        off += sz
    return blks


def build_nc(C):
    assert C % 16 == 0 and C <= T
    blks = _blocks(C)

    nc = bacc.Bacc(
        "TRN2",
        target_bir_lowering=False,
        debug=False,
        num_devices=NCORES,
    )

    xgs = [
        nc.dram_tensor(f"xg{i}", [128, HC, sz], f16, kind="ExternalInput")
        for i, (_, sz) in enumerate(blks)
    ]
    cb = nc.dram_tensor("cb", [C], f32, kind="ExternalInput")
    w1p = nc.dram_tensor("w1p", [FC, 128, HC, 128], f16, kind="ExternalInput")
    w3p = nc.dram_tensor("w3p", [FC, 128, HC, 128], f16, kind="ExternalInput")
    w2p = nc.dram_tensor("w2p", [HC, 2, 128, FC // 2, 128], f16, kind="ExternalInput")
    out = nc.dram_tensor("out", [H, C], f16, kind="ExternalOutput")

    AF = mybir.ActivationFunctionType

    with tile.TileContext(nc) as tc:
        with (
            tc.tile_pool(name="big", bufs=1) as big,
            tc.tile_pool(name="singles", bufs=1) as singles,
            tc.tile_pool(name="wpool", bufs=10) as wpool,
            tc.tile_pool(name="w2pool", bufs=1) as w2pool,
            tc.tile_pool(name="evict", bufs=3) as evict,
            tc.tile_pool(name="psum_w", bufs=2, space="PSUM") as psum_w,
            tc.tile_pool(name="psum_gu", bufs=2, space="PSUM") as psum_gu,
            tc.tile_pool(name="psum_o", bufs=2, space="PSUM") as psum_o,
        ):
            # ---- PE warmup: ramp the p-state while inputs stream in ----
            warm = singles.tile([128, 512], f16, tag="warm")
            nc.vector.memset(warm, 0.0)
            # one long accumulation chain: back-to-back matmuls with no
            # inter-instruction semaphore waits, so the PE p-state ramps
            # continuously and real work starts at full clock
            wps = psum_w.tile([128, 512], f32, tag="wm")
            for i in range(NWARM):
                nc.tensor.matmul(wps, lhsT=warm[:, :128], rhs=warm,
                                 start=(i == 0), stop=(i == NWARM - 1))

            # ---- input loads ----
            # SP ring: x block 0, then the just-in-time w1/w3 stream.
            # ACT ring: x block 1+, cb, then the whole-w2 preload.
            xb = []
            for i, (_, sz) in enumerate(blks):
                t = singles.tile([128, HC, sz], f16, tag=f"xb{i}")
                eng = nc.sync if i == 0 else nc.scalar
                eng.dma_start(out=t, in_=xgs[i].ap())
                xb.append(t)
            cb_b = singles.tile([128, C], f32, tag="cb")
            cb_src = bass.AP(tensor=cb.ap().tensor, offset=0, ap=[[0, 128], [1, C]])
            nc.scalar.dma_start(out=cb_b, in_=cb_src)

            # ---- phase 1: gus = gelu(w1 @ x) * (w3 @ x), [F-part, tokens] ----
            # The w2 preload is paced through the f-loop (one 512KB chunk per
            # two f-iters on the ACT ring) so its burst can't starve the
            # just-in-time w1/w3 stream of DMA-engine bandwidth.
            w2sb = {}
            w2chunks = [(h, hv) for h in range(HC) for hv in range(2)]
            gus = big.tile([128, FC, C], f16)
            for f in range(FC):
                w1f = wpool.tile([128, HC, 128], f16, tag="w1f")
                nc.sync.dma_start(out=w1f, in_=w1p.ap()[f])
                w3f = wpool.tile([128, HC, 128], f16, tag="w3f")
                nc.scalar.dma_start(out=w3f, in_=w3p.ap()[f])
                if f % 2 == 0 and f // 2 < len(w2chunks):
                    h2, hv2 = w2chunks[f // 2]
                    t = w2pool.tile([128, FC // 2, 128], f16, tag=f"w2_{h2}_{hv2}")
                    nc.scalar.dma_start(out=t, in_=w2p.ap()[h2, hv2])
                    w2sb[h2, hv2] = t
                for bi, (off, sz) in enumerate(blks):
                    g_ps = psum_gu.tile([128, 512], f32, tag="g")
                    for h in range(HC):
                        nc.tensor.matmul(
                            g_ps[:, :sz],
                            lhsT=w1f[:, h, :],
                            rhs=xb[bi][:, h, :],
                            start=(h == 0),
                            stop=(h == HC - 1),
                        )
                    u_ps = psum_gu.tile([128, 512], f32, tag="u")
                    for h in range(HC):
                        nc.tensor.matmul(
                            u_ps[:, :sz],
                            lhsT=w3f[:, h, :],
                            rhs=xb[bi][:, h, :],
                            start=(h == 0),
                            stop=(h == HC - 1),
                        )
                    gs = evict.tile([128, 512], f32, tag="gs")
                    nc.scalar.activation(gs[:, :sz], g_ps[:, :sz], AF.Gelu)
                    nc.vector.tensor_mul(
                        gus[:, f, off : off + sz], gs[:, :sz], u_ps[:, :sz]
                    )

            # ---- phase 2: out[h,:] = (w2 @ gus) * cb (w2 already in SBUF) ----
            for h in range(HC):
                for off, sz in blks:
                    o_ps = psum_o.tile([128, 512], f32, tag="o")
                    for f in range(FC):
                        nc.tensor.matmul(
                            o_ps[:, :sz],
                            lhsT=w2sb[h, f // (FC // 2)][:, f % (FC // 2), :],
                            rhs=gus[:, f, off : off + sz],
                            start=(f == 0),
                            stop=(f == FC - 1),
                        )
                    o_sb = evict.tile([128, 512], f16, tag="osb")
                    # split the last iteration's evict so the final out-DMA
                    # overlaps the final multiply instead of chaining after it
                    halves = (
                        [(0, sz // 2), (sz // 2, sz - sz // 2)]
                        if (h == HC - 1 and (off, sz) == blks[-1])
                        else [(0, sz)]
                    )
                    for hk, (ho, hs_) in enumerate(halves):
                        nc.vector.tensor_mul(
                            o_sb[:, ho : ho + hs_],
                            o_ps[:, ho : ho + hs_],
                            cb_b[:, off + ho : off + ho + hs_],
                        )
                        eng = nc.scalar if hk == 1 else nc.sync
                        eng.dma_start(
                            out=out.ap()[ts(h, 128), off + ho : off + ho + hs_],
                            in_=o_sb[:, ho : ho + hs_],
                        )
    nc.finalize()
    return nc


def _route(hidden_states, gate_w):
    """Exact fp32 router matching the reference: softcap -> softmax -> top2."""
    hs = hidden_states.astype(np.float32)
    logits = hs @ gate_w.T.astype(np.float32)
    logits = 30.0 * np.tanh(logits / 30.0)
    lmax = logits.max(axis=-1, keepdims=True)
    p = np.exp(logits - lmax)
    probs = p / p.sum(axis=-1, keepdims=True)
    idx = np.argsort(-probs, axis=-1, kind="stable")[:, :2]
    vals = np.take_along_axis(probs, idx, axis=-1)
    return idx, vals


def _prep_weights(w1, w2, w3):
    """Pre-transpose expert weights into contiguous SBUF tile layouts."""
    wmaps = []
    for e in range(NCORES):
        # w1p[f, p, c, m] = w1[e][f*128+m, c*128+p]
        w1p = np.ascontiguousarray(
            w1[e].astype(np.float16).reshape(FC, 128, HC, 128).transpose(0, 3, 2, 1)
        )
        w3p = np.ascontiguousarray(
            w3[e].astype(np.float16).reshape(FC, 128, HC, 128).transpose(0, 3, 2, 1)
        )
        # w2p[h, hv, p, c, m] = w2[e][h*128+m, hv*F/2 + c*128+p]
        w2p = np.ascontiguousarray(
            w2[e]
            .astype(np.float16)
            .reshape(HC, 128, 2, FC // 2, 128)
            .transpose(0, 2, 4, 3, 1)
        )
        wmaps.append({"w1p": w1p, "w3p": w3p, "w2p": w2p})
    return wmaps


def kernel(hidden_states, gate_w, w1, w2, w3, trace=False):
    hidden_states = np.asarray(hidden_states, dtype=np.float32)
    gate_w = np.asarray(gate_w, dtype=np.float32)
    w1 = np.asarray(w1, dtype=np.float32)
    w2 = np.asarray(w2, dtype=np.float32)
    w3 = np.asarray(w3, dtype=np.float32)

    idx, vals = _route(hidden_states, gate_w)
    toks = []
    cvals = []
    for e in range(E):
        hit = idx == e                                     # [T, 2]
        tok_e = np.where(hit.any(axis=1))[0]
        toks.append(tok_e)
        cvals.append((vals * hit)[tok_e].sum(axis=1).astype(np.float32))
    nmax = max(len(t) for t in toks)
    C = max(16, -(-nmax // 16) * 16)
    blks = _blocks(C)

    if C not in _CACHE:
        _CACHE[C] = build_nc(C)
    nc = _CACHE[C]

    xT16 = np.ascontiguousarray(hidden_states.T).astype(np.float16)  # [H, T]
    wmaps = _prep_weights(w1, w2, w3)
    in_maps = []
    for e in range(NCORES):
        n_e = len(toks[e])
        xg = np.zeros((128, HC, C), dtype=np.float16)
        # xg[p, c, :n] = x[c*128+p, toks]
        xg[:, :, :n_e] = xT16.reshape(HC, 128, T)[:, :, toks[e]].transpose(1, 0, 2)
        cbv = np.zeros((C,), dtype=np.float32)
        cbv[:n_e] = cvals[e]
        m = {"cb": cbv, **wmaps[e]}
        for i, (off, sz) in enumerate(blks):
            m[f"xg{i}"] = np.ascontiguousarray(xg[:, :, off : off + sz])
        in_maps.append(m)

    res = run_bass_kernel_spmd(nc, in_maps, core_ids=list(range(NCORES)), trace=trace)
    out = np.zeros((T, H), dtype=np.float32)
    for e in range(NCORES):
        n_e = len(toks[e])
        out[toks[e]] += res.results[e]["out"][:, :n_e].astype(np.float32).T
    _CACHE["last_results"] = res
    return out


if __name__ == "__main__":
    rng = np.random.default_rng(0)
    hs = rng.standard_normal((T, H), dtype=np.float32)
    gw = (rng.standard_normal((E, H)) * 0.02).astype(np.float32)
    w1 = (rng.standard_normal((E, F, H)) * 0.02).astype(np.float32)
    w2 = (rng.standard_normal((E, H, F)) * 0.02).astype(np.float32)
    w3 = (rng.standard_normal((E, F, H)) * 0.02).astype(np.float32)
    out = kernel(hs, gw, w1, w2, w3)
    print("out", out.shape, out.dtype, np.abs(out).max())
